# revision 28
# baseline (speedup 1.0000x reference)
"""Trainium2 Bass kernel for the hybrid attention head (nn_AttentionHead_Hybrid).

Math (per batch):
    norms  n_i = ||x_i||;  xh = x / n
    O      = product of 2016 Givens rotations (built on host, fp32)
    S[i,j] = xh_i . O . xh_j
    A      = S^2 * n_i n_j ;  P = softmax(A / 8)
    V      = x @ Vw^T + Vb
    out    = LayerNorm(P @ V + x) * gamma + beta

Device formulation (per core, 4 batches):
    W      = diag(s) X with s_n = ||x_n||^-1/2 * 8^-1/4 * C^1/4, C = 1024*log2(e)
    G      = O^T W^T (host-prepped f16, like W)
    R      = W G                  ->  R[j,i]^2 = C * A[i,j]/8
    asq    = Square(R)  (f16)     one ACT/DVE pass over the NxN matrix
    E^T    = f16-bits(asq + 15360)   Schraudolph: bitwise 2^z, one 4x-mode DVE add
    Vt     = [X Vw^T + Vb | 1 | rowmean] (host-prepped f16; extra cols give softmax
             row-sums and sum_d OUT / 64 for free via the PV matmul)
    POT    = sum_j Vt[j,:]^T E^T[j,:]   in [66, N] psum, PE-transposed back
    y      = OUT + rowsum * x     (softmax denom folded into LN scale invariance)
    out    = (y - mean) * rsqrt(var)   via Quake rsqrt + 1 Newton step

Scheduling: every engine queue is strict FIFO, so emission order is the
schedule. The j-loop is software-pipelined (pr matmuls run 2 tiles ahead of
PV matmuls) and the previous batch's epilogue is emitted in stages spread
across the next batch's j-loop, each stage landing in its engine FIFO only
once its dependencies are nearly complete (no head-of-line blocking).

Sharding: data-parallel over batch, 4 batches per core on 8 cores; params replicated.
"""

import math

import numpy as np

import concourse.bacc as bacc
import concourse.bass as bass
import concourse.tile as tile
from concourse import bass_utils, mybir

AF = mybir.ActivationFunctionType
ALU = mybir.AluOpType
DT = mybir.dt

B, N, D = 32, 1024, 64
NCORES = 8
BPC = B // NCORES          # batches per core
NT = N // 128              # 128-row tiles per batch
DP = D + 2                 # V dims + rowsum col + rowmean col

CSCALE = 1024.0 * math.log2(math.e)   # f16 Schraudolph exponent scale
BITS_BIAS = 15360.0                    # 15 << 10 (f16 exponent bias field)

# j-tiles whose square is computed on DVE instead of ACT (engine balance)
DVE_SQ = frozenset()


def _build_orthogonal(phi: np.ndarray, d: int = D) -> np.ndarray:
    """fp32 replica of the reference jax.lax.scan Givens chain."""
    O = np.eye(d, dtype=np.float32)
    ii, jj = np.triu_indices(d, k=1)
    c = np.cos(phi.astype(np.float32))
    s = np.sin(phi.astype(np.float32))
    for k in range(len(phi)):
        i, j = int(ii[k]), int(jj[k])
        ri = O[i].copy()
        rj = O[j].copy()
        O[i] = c[k] * ri + s[k] * rj
        O[j] = -s[k] * ri + c[k] * rj
    return O


def _build_nc(apply_gamma_beta: bool):
    nc = bacc.Bacc("TRN2", target_bir_lowering=False)

    xz_t = nc.dram_tensor("xz", [BPC, N, D + 1], DT.float32, kind="ExternalInput")
    gw_t = nc.dram_tensor("gw", [BPC, 128, 2 * N], DT.float16, kind="ExternalInput")
    v_t = nc.dram_tensor("v", [BPC, N, DP], DT.float16, kind="ExternalInput")
    id_t = nc.dram_tensor("ident", [128, 128], DT.float32, kind="ExternalInput")
    gb_t = nc.dram_tensor("gb", [2, D], DT.float32, kind="ExternalInput")
    out_t = nc.dram_tensor("out", [BPC, N, D], DT.float16, kind="ExternalOutput")

    with tile.TileContext(nc) as tc:
        with (
            tc.tile_pool(name="const", bufs=1) as constp,
            tc.tile_pool(name="xp", bufs=2) as xp,
            tc.tile_pool(name="wtp", bufs=2) as wtp,
            tc.tile_pool(name="gp", bufs=2) as gp,
            tc.tile_pool(name="ep", bufs=4) as ep,
            tc.tile_pool(name="vp", bufs=2) as vp,
            tc.tile_pool(name="sqp", bufs=4) as sqp,
            tc.tile_pool(name="otp", bufs=2) as otp,
            tc.tile_pool(name="yp", bufs=2) as yp,
            tc.tile_pool(name="statp", bufs=2) as statp,
            tc.tile_pool(name="ps_r", bufs=3, space="PSUM") as ps_r,
            tc.tile_pool(name="ps_ot", bufs=2, space="PSUM") as ps_ot,
        ):
            id_sb = constp.tile([128, 128], DT.float32)
            nc.sync.dma_start(out=id_sb, in_=id_t[:, :])
            if apply_gamma_beta:
                gam_sb = constp.tile([128, D], DT.float32)
                nc.sync.dma_start(out=gam_sb, in_=gb_t[0, :].to_broadcast([128, D]))
                bet_sb = constp.tile([128, D], DT.float32)
                nc.sync.dma_start(out=bet_sb, in_=gb_t[1, :].to_broadcast([128, D]))

            # PE warm-up: dependency-free matmuls trip the HAM activity
            # window so real matmuls run at 2.4 GHz, not 1.2.
            warm = constp.tile([64, 512], DT.float16)
            nc.vector.memset(warm, 0.0)
            pw = ps_r.tile([128, N], DT.float32, tag="r")
            for _ in range(8):
                nc.tensor.matmul(pw[0:64, 0:512], lhsT=warm[:, 0:64], rhs=warm)

            def emit_loads(b):
                gw = gp.tile([128, 2 * N], DT.float16, tag="gw")
                nc.sync.dma_start(out=gw[0:D, :], in_=gw_t[b, 0:D])
                nc.sync.dma_start(out=gw[D : 2 * D, :], in_=gw_t[b, D : 2 * D])
                g_sb = gw[:, 0:N]
                wt = gw[:, N : 2 * N]
                v_sb = vp.tile([128, NT, DP], DT.float16, tag="v")
                nc.sync.dma_start(
                    out=v_sb, in_=v_t[b].rearrange("(t p) f -> p t f", p=128)
                )
                xz = xp.tile([128, NT, D + 1], DT.float32, tag="xz")
                nc.sync.dma_start(
                    out=xz, in_=xz_t[b].rearrange("(t p) d -> p t d", p=128)
                )
                x_sb = xz[:, :, 0:D]
                xs_sb = xz[:, :, D]
                return x_sb, xs_sb, wt, g_sb, v_sb

            state = {0: emit_loads(0)}
            pots = {}
            epi = {}
            e_tiles = {}

            def ep_ot(b):
                """stage 1: drain POT to SBUF (frees the psum accumulator)."""
                pot_a, pot_b = pots.pop(b)
                ot_sb = otp.tile([DP, N], DT.float32, tag="ot")
                if b == BPC - 1:
                    # tail is DVE-bound: keep DVE free, ACT is idle here
                    nc.scalar.copy(ot_sb[:, 0:512], pot_a)
                    nc.scalar.copy(ot_sb[:, 512:N], pot_b)
                else:
                    nc.scalar.copy(ot_sb[:, 0:512], pot_a)
                    nc.vector.tensor_copy(ot_sb[:, 512:N], pot_b)
                epi[b] = {"ot": ot_sb}

            def ep_transpose(b):
                """stage 2: transpose back, y = OUT + rowsum * x."""
                x_sb, xs_sb, wt, g_sb, v_sb = state[b]
                st = epi[b]
                ot_sb = st["ot"]
                y_sb = yp.tile([128, NT, D], DT.float32, tag="y")
                ros = statp.tile([128, NT, 2], DT.float32, tag="ros")
                for grp in range(2):
                    ptr = ps_r.tile([128, 4, DP], DT.float32, tag="r")
                    for q in range(4):
                        it = grp * 4 + q
                        nc.tensor.transpose(
                            ptr[:, q, :],
                            ot_sb[:, it * 128 : (it + 1) * 128],
                            id_sb[0:DP, 0:DP],
                        )
                    g_sl = slice(grp * 4, grp * 4 + 4)
                    nc.vector.tensor_copy(ros[:, g_sl, :], ptr[:, :, D : D + 2])
                    rs4 = ros[:, g_sl, 0]
                    rs_bc = bass.AP(
                        tensor=ros.tensor, offset=rs4.offset,
                        ap=[ros.ap[0], [2, 4], [0, D]],
                    )
                    nc.vector.tensor_tensor(
                        out=y_sb[:, g_sl, :], in0=x_sb[:, g_sl, :], in1=rs_bc,
                        op=ALU.mult,
                    )
                    nc.vector.tensor_add(
                        y_sb[:, g_sl, :], y_sb[:, g_sl, :], ptr[:, :, 0:D]
                    )
                st["y"] = y_sb
                st["ros"] = ros

            def ep_stats(b):
                """stage 3: LN stats (mean via extra PV col, var via square)."""
                x_sb, xs_sb, wt, g_sb, v_sb = state[b]
                st = epi[b]
                y_sb, ros = st["y"], st["ros"]
                mean = statp.tile([128, NT], DT.float32, tag="mean")
                nc.vector.tensor_mul(mean, ros[:, :, 0], xs_sb)
                nc.vector.tensor_add(mean, mean, ros[:, :, 1])
                ysq = statp.tile([128, NT, D], DT.float32, tag="ysq")
                nc.gpsimd.tensor_mul(ysq, y_sb, y_sb)
                var = statp.tile([128, NT], DT.float32, tag="var")
                nc.vector.reduce_sum(var, ysq, axis=mybir.AxisListType.X)
                nc.vector.tensor_scalar_mul(var, var, 1.0 / D)
                msq = statp.tile([128, NT], DT.float32, tag="msq")
                nc.vector.tensor_mul(msq, mean, mean)
                nc.vector.tensor_sub(var, var, msq)
                st["mean"] = mean
                st["var"] = var

            def ep_norm(b):
                """stage 4: Quake rsqrt + normalize + store."""
                st = epi.pop(b)
                y_sb, mean, var = st["y"], st["mean"], st["var"]
                rstd = statp.tile([128, NT], DT.float32, tag="rstd")
                iv = statp.tile([128, NT], DT.int32, tag="iv")
                nc.vector.tensor_scalar(
                    iv, var.bitcast(DT.int32), scalar1=1, scalar2=None,
                    op0=ALU.arith_shift_right,
                )
                nc.vector.tensor_scalar(
                    iv, iv, scalar1=-1, scalar2=None, op0=ALU.bitwise_xor
                )
                nc.vector.tensor_scalar_add(iv, iv, 0x5F3759E0)
                yk = iv.bitcast(DT.float32)
                t1 = statp.tile([128, NT], DT.float32, tag="t1")
                nc.vector.tensor_mul(t1, yk, yk)
                nc.vector.tensor_mul(t1, t1, var)
                nc.vector.tensor_scalar(
                    t1, t1, scalar1=-0.5, scalar2=1.5,
                    op0=ALU.mult, op1=ALU.add,
                )
                nc.vector.tensor_mul(rstd, yk, t1)
                mean_bc = bass.AP(
                    tensor=mean.tensor, offset=mean.offset,
                    ap=[mean.ap[0], [1, NT], [0, D]],
                )
                rstd_bc = bass.AP(
                    tensor=rstd.tensor, offset=rstd.offset,
                    ap=[rstd.ap[0], [1, NT], [0, D]],
                )
                nc.vector.tensor_tensor(out=y_sb, in0=y_sb, in1=mean_bc, op=ALU.subtract)
                y16 = yp.tile([128, NT, D], DT.float16, tag="y16")
                if apply_gamma_beta:
                    nc.vector.tensor_tensor(out=y_sb, in0=y_sb, in1=rstd_bc, op=ALU.mult)
                    for it in range(NT):
                        nc.gpsimd.tensor_mul(y_sb[:, it, :], y_sb[:, it, :], gam_sb)
                        nc.gpsimd.tensor_add(y_sb[:, it, :], y_sb[:, it, :], bet_sb)
                    nc.vector.tensor_copy(y16, y_sb)
                else:
                    nc.vector.tensor_tensor(out=y16, in0=y_sb, in1=rstd_bc, op=ALU.mult)
                nc.sync.dma_start(
                    out=out_t[b].rearrange("(t p) d -> p t d", p=128), in_=y16
                )

            TOTAL = BPC * NT
            for T in range(TOTAL + 2):
                if T < TOTAL:
                    b, jt = divmod(T, NT)
                    x_sb, xs_sb, wt, g_sb, v_sb = state[b]
                    asq = sqp.tile([128, N], DT.float16, tag="asq")
                    pr = ps_r.tile([128, N], DT.float32, tag="r")
                    for h in range(2):
                        nc.tensor.matmul(
                            pr[:, h * 512 : (h + 1) * 512],
                            lhsT=wt[h * D : (h + 1) * D,
                                    jt * 128 : (jt + 1) * 128],
                            rhs=g_sb[h * D : (h + 1) * D,
                                     h * 512 : (h + 1) * 512],
                            tile_position=(h * 64, 0),
                        )
                    if jt in DVE_SQ:
                        rf = sqp.tile([128, N], DT.float16, tag="rf")
                        nc.vector.tensor_copy(rf, pr)
                        nc.vector.tensor_mul(asq, rf, rf)
                    else:
                        nc.scalar.activation(asq, pr, AF.Square)
                    # Schraudolph: f16 bits = round(asq + 15360) -> 2^z
                    e_sb = ep.tile([128, N], DT.int16, tag="e")
                    nc.vector.tensor_scalar(
                        out=e_sb, in0=asq, scalar1=BITS_BIAS, scalar2=None,
                        op0=ALU.add,
                    )
                    e_tiles[T] = (e_sb, None)
                    if jt == 2 and b + 1 < BPC:
                        state[b + 1] = emit_loads(b + 1)
                Tb = T - 2
                if Tb >= 0:
                    b2, jt2 = divmod(Tb, NT)
                    if jt2 == 0:
                        pot_a = ps_ot.tile([DP, 512], DT.float32, tag="ot", name=f"potA{b2}")
                        pot_b = ps_ot.tile([DP, 512], DT.float32, tag="ot", name=f"potB{b2}")
                        pots[b2] = (pot_a, pot_b)
                    pot_pair = pots[b2]
                    e_pr, _ = e_tiles.pop(Tb)
                    v_sb2 = state[b2][4]
                    for c in range(2):
                        nc.tensor.matmul(
                            pot_pair[c],
                            lhsT=v_sb2[:, jt2, :],
                            rhs=e_pr[:, c * 512 : (c + 1) * 512].bitcast(
                                DT.float16
                            ),
                            start=(jt2 == 0),
                            stop=(jt2 == NT - 1),
                        )
                    if jt2 == NT - 1:
                        ep_ot(b2)
                    if b2 >= 1:
                        if jt2 == 1:
                            ep_transpose(b2 - 1)
                        elif jt2 == 3:
                            ep_stats(b2 - 1)
                        elif jt2 == 5:
                            ep_norm(b2 - 1)

            # last batch: per-group staged epilogue to shorten the tail chain
            bL = BPC - 1
            x_sb, xs_sb, wt, g_sb, v_sb = state[bL]
            st = epi[bL]
            ot_sb = st["ot"]
            y_sb = yp.tile([128, NT, D], DT.float32, tag="y")
            ros = statp.tile([128, NT, 2], DT.float32, tag="ros")
            mean = statp.tile([128, NT], DT.float32, tag="mean")
            ysq = statp.tile([128, NT, D], DT.float32, tag="ysq")
            var = statp.tile([128, NT], DT.float32, tag="var")
            msq = statp.tile([128, NT], DT.float32, tag="msq")
            rstd = statp.tile([128, NT], DT.float32, tag="rstd")
            iv = statp.tile([128, NT], DT.int32, tag="iv")
            t1 = statp.tile([128, NT], DT.float32, tag="t1")
            for grp in range(2):
                g_sl = slice(grp * 4, grp * 4 + 4)
                ptr = ps_r.tile([128, 4, DP], DT.float32, tag="r")
                for q in range(4):
                    it = grp * 4 + q
                    nc.tensor.transpose(
                        ptr[:, q, :],
                        ot_sb[:, it * 128 : (it + 1) * 128],
                        id_sb[0:DP, 0:DP],
                    )
                nc.vector.tensor_copy(ros[:, g_sl, :], ptr[:, :, D : D + 2])
                rs4 = ros[:, g_sl, 0]
                rs_bc = bass.AP(
                    tensor=ros.tensor, offset=rs4.offset,
                    ap=[ros.ap[0], [2, 4], [0, D]],
                )
                nc.vector.tensor_tensor(
                    out=y_sb[:, g_sl, :], in0=x_sb[:, g_sl, :], in1=rs_bc,
                    op=ALU.mult,
                )
                nc.vector.tensor_add(
                    y_sb[:, g_sl, :], y_sb[:, g_sl, :], ptr[:, :, 0:D]
                )
                nc.vector.tensor_mul(mean[:, g_sl], ros[:, g_sl, 0], xs_sb[:, g_sl])
                nc.vector.tensor_add(mean[:, g_sl], mean[:, g_sl], ros[:, g_sl, 1])
                nc.scalar.activation(ysq[:, g_sl, :], y_sb[:, g_sl, :], AF.Square)
                nc.vector.reduce_sum(
                    var[:, g_sl], ysq[:, g_sl, :], axis=mybir.AxisListType.X
                )
                nc.vector.tensor_scalar_mul(var[:, g_sl], var[:, g_sl], 1.0 / D)
                nc.vector.tensor_mul(msq[:, g_sl], mean[:, g_sl], mean[:, g_sl])
                nc.vector.tensor_sub(var[:, g_sl], var[:, g_sl], msq[:, g_sl])
                nc.vector.tensor_scalar(
                    iv[:, g_sl], var[:, g_sl].bitcast(DT.int32), scalar1=1,
                    scalar2=None, op0=ALU.arith_shift_right,
                )
                nc.vector.tensor_scalar(
                    iv[:, g_sl], iv[:, g_sl], scalar1=-1, scalar2=None,
                    op0=ALU.bitwise_xor,
                )
                nc.vector.tensor_scalar_add(iv[:, g_sl], iv[:, g_sl], 0x5F3759E0)
                yk = iv.bitcast(DT.float32)
                nc.vector.tensor_mul(t1[:, g_sl], yk[:, g_sl], yk[:, g_sl])
                nc.vector.tensor_mul(t1[:, g_sl], t1[:, g_sl], var[:, g_sl])
                nc.vector.tensor_scalar(
                    t1[:, g_sl], t1[:, g_sl], scalar1=-0.5, scalar2=1.5,
                    op0=ALU.mult, op1=ALU.add,
                )
                nc.vector.tensor_mul(rstd[:, g_sl], yk[:, g_sl], t1[:, g_sl])
                m4 = mean[:, g_sl]
                mean_bc = bass.AP(
                    tensor=mean.tensor, offset=m4.offset,
                    ap=[mean.ap[0], [1, 4], [0, D]],
                )
                r4 = rstd[:, g_sl]
                rstd_bc = bass.AP(
                    tensor=rstd.tensor, offset=r4.offset,
                    ap=[rstd.ap[0], [1, 4], [0, D]],
                )
                nc.vector.tensor_tensor(
                    out=y_sb[:, g_sl, :], in0=y_sb[:, g_sl, :], in1=mean_bc,
                    op=ALU.subtract,
                )
                if grp == 0:
                    y16t = yp.tile([128, NT, D], DT.float16, tag="y16")
                if apply_gamma_beta:
                    nc.vector.tensor_tensor(
                        out=y_sb[:, g_sl, :], in0=y_sb[:, g_sl, :], in1=rstd_bc,
                        op=ALU.mult,
                    )
                    for it in range(grp * 4, grp * 4 + 4):
                        nc.gpsimd.tensor_mul(y_sb[:, it, :], y_sb[:, it, :], gam_sb)
                        nc.gpsimd.tensor_add(y_sb[:, it, :], y_sb[:, it, :], bet_sb)
                    nc.vector.tensor_copy(y16t[:, g_sl, :], y_sb[:, g_sl, :])
                else:
                    nc.vector.tensor_tensor(
                        out=y16t[:, g_sl, :], in0=y_sb[:, g_sl, :], in1=rstd_bc,
                        op=ALU.mult,
                    )
                nc.sync.dma_start(
                    out=out_t[bL]
                    .rearrange("(t p) d -> p t d", p=128)[:, g_sl, :],
                    in_=y16t[:, g_sl, :],
                )


    nc.compile()
    return nc


_NC_CACHE: dict = {}


def kernel(input1, V_w, V_b, phi, ln_gamma, ln_beta, _trace=False):
    input1 = np.ascontiguousarray(np.asarray(input1, dtype=np.float32))
    V_w = np.asarray(V_w, dtype=np.float32)
    V_b = np.asarray(V_b, dtype=np.float32)
    phi = np.asarray(phi, dtype=np.float32)
    ln_gamma = np.asarray(ln_gamma, dtype=np.float32)
    ln_beta = np.asarray(ln_beta, dtype=np.float32)

    apply_gb = not (np.all(ln_gamma == 1.0) and np.all(ln_beta == 0.0))

    if apply_gb not in _NC_CACHE:
        _NC_CACHE[apply_gb] = _build_nc(apply_gb)
    nc = _NC_CACHE[apply_gb]

    O = _build_orthogonal(phi)
    ident = np.eye(128, dtype=np.float32)
    gb = np.ascontiguousarray(np.stack([ln_gamma, ln_beta]).astype(np.float32))

    # host prep: scaled rows W = x * ||x||^-1/2 * 8^-1/4 * C^1/4, transposed;
    # G = O^T W^T; Vt = [X Vw^T + Vb | 1 | rowmean]; per-row sums / D
    ss = (input1.astype(np.float64) ** 2).sum(-1)
    s = (ss ** -0.25 * 8.0 ** -0.25 * CSCALE ** 0.25).astype(np.float32)
    w = input1 * s[..., None]
    w16 = w.astype(np.float16)
    wt_full = w16.transpose(0, 2, 1)
    o16f = O.astype(np.float16).astype(np.float32)
    g_full = np.einsum(
        "bnd,de->ben", w16.astype(np.float32), o16f, optimize=True
    ).astype(np.float16)
    top = np.concatenate([g_full, wt_full], axis=2)          # [B, 64, 2N]
    gw_full = np.ascontiguousarray(np.concatenate([top, top], axis=1))
    v_full = np.empty((B, N, DP), np.float32)
    v_full[:, :, 0:D] = input1 @ V_w.T + V_b
    v_full[:, :, D] = 1.0
    v_full[:, :, D + 1] = v_full[:, :, 0:D].sum(-1) / D
    v_full = np.ascontiguousarray(v_full.astype(np.float16))
    xz_full = np.empty((B, N, D + 1), np.float32)
    xz_full[:, :, 0:D] = input1
    xz_full[:, :, D] = input1.sum(-1) / D
    xz_full = np.ascontiguousarray(xz_full)

    in_maps = []
    for c in range(NCORES):
        sl = slice(c * BPC, (c + 1) * BPC)
        in_maps.append(
            {
                "xz": xz_full[sl],
                "gw": gw_full[sl],
                "v": v_full[sl],
                "ident": ident,
                "gb": gb,
            }
        )

    res = bass_utils.run_bass_kernel_spmd(
        nc, in_maps, core_ids=list(range(NCORES)), trace=_trace
    )
    out = np.concatenate([res.results[c]["out"] for c in range(NCORES)], axis=0).astype(np.float32)
    if _trace:
        kernel._last_result = res
    return out


# revision 29
# speedup vs baseline: 1.0479x; 1.0479x over previous
"""Trainium2 Bass kernel for the hybrid attention head (nn_AttentionHead_Hybrid).

Math (per batch):
    norms  n_i = ||x_i||;  xh = x / n
    O      = product of 2016 Givens rotations (built on host, fp32)
    S[i,j] = xh_i . O . xh_j
    A      = S^2 * n_i n_j ;  P = softmax(A / 8)
    V      = x @ Vw^T + Vb
    out    = LayerNorm(P @ V + x) * gamma + beta

Device formulation (per core, 4 batches):
    W      = diag(s) X with s_n = ||x_n||^-1/2 * 8^-1/4 * C^1/4, C = 1024*log2(e)
    G      = O^T W^T (host-prepped f16, like W)
    R      = W G                  ->  R[j,i]^2 = C * A[i,j]/8
    asq    = Square(R)  (f16)     one ACT/DVE pass over the NxN matrix
    E^T    = f16-bits(asq + 15360)   Schraudolph: bitwise 2^z, one 4x-mode DVE add
    Vt     = [X Vw^T + Vb | 1 | rowmean] (host-prepped f16; extra cols give softmax
             row-sums and sum_d OUT / 64 for free via the PV matmul)
    POT    = sum_j Vt[j,:]^T E^T[j,:]   in [66, N] psum, PE-transposed back
    y      = OUT + rowsum * x     (softmax denom folded into LN scale invariance)
    out    = (y - mean) * rsqrt(var)   via Quake rsqrt + 1 Newton step

Scheduling: every engine queue is strict FIFO, so emission order is the
schedule. The j-loop is software-pipelined (pr matmuls run 2 tiles ahead of
PV matmuls) and the previous batch's epilogue is emitted in stages spread
across the next batch's j-loop, each stage landing in its engine FIFO only
once its dependencies are nearly complete (no head-of-line blocking).

Sharding: data-parallel over batch, 4 batches per core on 8 cores; params replicated.
"""

import math

import numpy as np

import concourse.bacc as bacc
import concourse.bass as bass
import concourse.tile as tile
from concourse import bass_utils, mybir

AF = mybir.ActivationFunctionType
ALU = mybir.AluOpType
DT = mybir.dt

B, N, D = 32, 1024, 64
NCORES = 8
BPC = B // NCORES          # batches per core
NT = N // 128              # 128-row tiles per batch
DP = D + 2                 # V dims + rowsum col + rowmean col

CSCALE = 1024.0 * math.log2(math.e)   # f16 Schraudolph exponent scale
BITS_BIAS = 15360.0                    # 15 << 10 (f16 exponent bias field)

# j-tiles whose square is computed on DVE instead of ACT (engine balance)
DVE_SQ = frozenset()


def _build_orthogonal(phi: np.ndarray, d: int = D) -> np.ndarray:
    """fp32 replica of the reference jax.lax.scan Givens chain."""
    O = np.eye(d, dtype=np.float32)
    ii, jj = np.triu_indices(d, k=1)
    c = np.cos(phi.astype(np.float32))
    s = np.sin(phi.astype(np.float32))
    for k in range(len(phi)):
        i, j = int(ii[k]), int(jj[k])
        ri = O[i].copy()
        rj = O[j].copy()
        O[i] = c[k] * ri + s[k] * rj
        O[j] = -s[k] * ri + c[k] * rj
    return O


def _build_nc(apply_gamma_beta: bool):
    nc = bacc.Bacc("TRN2", target_bir_lowering=False)

    xz_t = nc.dram_tensor("xz", [BPC, N, D + 1], DT.float32, kind="ExternalInput")
    gw_t = nc.dram_tensor("gw", [BPC, 128, 2 * N], DT.float16, kind="ExternalInput")
    v_t = nc.dram_tensor("v", [BPC, N, DP], DT.float16, kind="ExternalInput")
    id_t = nc.dram_tensor("ident", [128, 128], DT.float32, kind="ExternalInput")
    gb_t = nc.dram_tensor("gb", [2, D], DT.float32, kind="ExternalInput")
    out_t = nc.dram_tensor("out", [BPC, N, D], DT.float16, kind="ExternalOutput")

    with tile.TileContext(nc) as tc:
        with (
            tc.tile_pool(name="const", bufs=1) as constp,
            tc.tile_pool(name="xp", bufs=2) as xp,
            tc.tile_pool(name="wtp", bufs=2) as wtp,
            tc.tile_pool(name="gp", bufs=2) as gp,
            tc.tile_pool(name="ep", bufs=4) as ep,
            tc.tile_pool(name="vp", bufs=2) as vp,
            tc.tile_pool(name="sqp", bufs=4) as sqp,
            tc.tile_pool(name="otp", bufs=2) as otp,
            tc.tile_pool(name="yp", bufs=2) as yp,
            tc.tile_pool(name="statp", bufs=2) as statp,
            tc.tile_pool(name="ps_r", bufs=3, space="PSUM") as ps_r,
            tc.tile_pool(name="ps_ot", bufs=1, space="PSUM") as ps_ot,
        ):
            id_sb = constp.tile([128, 128], DT.float32)
            nc.sync.dma_start(out=id_sb, in_=id_t[:, :])
            if apply_gamma_beta:
                gam_sb = constp.tile([128, D], DT.float32)
                nc.sync.dma_start(out=gam_sb, in_=gb_t[0, :].to_broadcast([128, D]))
                bet_sb = constp.tile([128, D], DT.float32)
                nc.sync.dma_start(out=bet_sb, in_=gb_t[1, :].to_broadcast([128, D]))

            # PE warm-up: dependency-free matmuls trip the HAM activity
            # window so real matmuls run at 2.4 GHz, not 1.2.
            warm = constp.tile([64, 512], DT.float16)
            nc.vector.memset(warm, 0.0)
            pw = ps_r.tile([128, N], DT.float32, tag="r")
            for _ in range(8):
                nc.tensor.matmul(pw[0:64, 0:512], lhsT=warm[:, 0:64], rhs=warm)

            def emit_loads(b):
                gw = gp.tile([128, 2 * N], DT.float16, tag="gw")
                nc.sync.dma_start(out=gw[0:D, :], in_=gw_t[b, 0:D])
                nc.sync.dma_start(out=gw[D : 2 * D, :], in_=gw_t[b, D : 2 * D])
                g_sb = gw[:, 0:N]
                wt = gw[:, N : 2 * N]
                v_sb = vp.tile([128, NT, DP], DT.float16, tag="v")
                nc.sync.dma_start(
                    out=v_sb, in_=v_t[b].rearrange("(t p) f -> p t f", p=128)
                )
                xz = xp.tile([128, NT, D + 1], DT.float32, tag="xz")
                nc.sync.dma_start(
                    out=xz, in_=xz_t[b].rearrange("(t p) d -> p t d", p=128)
                )
                x_sb = xz[:, :, 0:D]
                xs_sb = xz[:, :, D]
                return x_sb, xs_sb, wt, g_sb, v_sb

            state = {0: emit_loads(0)}
            pots = {}
            epi = {}
            e_tiles = {}

            def ep_ot(b):
                """stage 1: drain POT to SBUF (frees the psum accumulator)."""
                pot = pots.pop(b)
                ot_sb = otp.tile([DP, N], DT.float32, tag="ot")
                if b == BPC - 1:
                    # tail is DVE-bound: keep DVE free, ACT is idle here
                    nc.scalar.copy(ot_sb[:, 0:512], pot[:, 0:512])
                    nc.scalar.copy(ot_sb[:, 512:N], pot[:, 512:N])
                else:
                    nc.scalar.copy(ot_sb[:, 0:512], pot[:, 0:512])
                    nc.vector.tensor_copy(ot_sb[:, 512:N], pot[:, 512:N])
                epi[b] = {"ot": ot_sb}

            def ep_transpose(b):
                """stage 2: transpose back, y = OUT + rowsum * x."""
                x_sb, xs_sb, wt, g_sb, v_sb = state[b]
                st = epi[b]
                ot_sb = st["ot"]
                y_sb = yp.tile([128, NT, D], DT.float32, tag="y")
                ros = statp.tile([128, NT, 2], DT.float32, tag="ros")
                for grp in range(2):
                    ptr = ps_r.tile([128, 4, DP], DT.float32, tag="r")
                    for q in range(4):
                        it = grp * 4 + q
                        nc.tensor.transpose(
                            ptr[:, q, :],
                            ot_sb[:, it * 128 : (it + 1) * 128],
                            id_sb[0:DP, 0:DP],
                        )
                    g_sl = slice(grp * 4, grp * 4 + 4)
                    nc.vector.tensor_copy(ros[:, g_sl, :], ptr[:, :, D : D + 2])
                    rs4 = ros[:, g_sl, 0]
                    rs_bc = bass.AP(
                        tensor=ros.tensor, offset=rs4.offset,
                        ap=[ros.ap[0], [2, 4], [0, D]],
                    )
                    nc.vector.tensor_tensor(
                        out=y_sb[:, g_sl, :], in0=x_sb[:, g_sl, :], in1=rs_bc,
                        op=ALU.mult,
                    )
                    nc.vector.tensor_add(
                        y_sb[:, g_sl, :], y_sb[:, g_sl, :], ptr[:, :, 0:D]
                    )
                st["y"] = y_sb
                st["ros"] = ros

            def ep_stats(b):
                """stage 3: LN stats (mean via extra PV col, var via square)."""
                x_sb, xs_sb, wt, g_sb, v_sb = state[b]
                st = epi[b]
                y_sb, ros = st["y"], st["ros"]
                mean = statp.tile([128, NT], DT.float32, tag="mean")
                nc.vector.tensor_mul(mean, ros[:, :, 0], xs_sb)
                nc.vector.tensor_add(mean, mean, ros[:, :, 1])
                ysq = statp.tile([128, NT, D], DT.float32, tag="ysq")
                nc.gpsimd.tensor_mul(ysq, y_sb, y_sb)
                var = statp.tile([128, NT], DT.float32, tag="var")
                nc.vector.reduce_sum(var, ysq, axis=mybir.AxisListType.X)
                nc.vector.tensor_scalar_mul(var, var, 1.0 / D)
                msq = statp.tile([128, NT], DT.float32, tag="msq")
                nc.vector.tensor_mul(msq, mean, mean)
                nc.vector.tensor_sub(var, var, msq)
                st["mean"] = mean
                st["var"] = var

            def ep_norm(b):
                """stage 4: Quake rsqrt + normalize + store."""
                st = epi.pop(b)
                y_sb, mean, var = st["y"], st["mean"], st["var"]
                rstd = statp.tile([128, NT], DT.float32, tag="rstd")
                iv = statp.tile([128, NT], DT.int32, tag="iv")
                nc.vector.tensor_scalar(
                    iv, var.bitcast(DT.int32), scalar1=1, scalar2=None,
                    op0=ALU.arith_shift_right,
                )
                nc.vector.tensor_scalar(
                    iv, iv, scalar1=-1, scalar2=None, op0=ALU.bitwise_xor
                )
                nc.vector.tensor_scalar_add(iv, iv, 0x5F3759E0)
                yk = iv.bitcast(DT.float32)
                t1 = statp.tile([128, NT], DT.float32, tag="t1")
                nc.vector.tensor_mul(t1, yk, yk)
                nc.vector.tensor_mul(t1, t1, var)
                nc.vector.tensor_scalar(
                    t1, t1, scalar1=-0.5, scalar2=1.5,
                    op0=ALU.mult, op1=ALU.add,
                )
                nc.vector.tensor_mul(rstd, yk, t1)
                mean_bc = bass.AP(
                    tensor=mean.tensor, offset=mean.offset,
                    ap=[mean.ap[0], [1, NT], [0, D]],
                )
                rstd_bc = bass.AP(
                    tensor=rstd.tensor, offset=rstd.offset,
                    ap=[rstd.ap[0], [1, NT], [0, D]],
                )
                nc.vector.tensor_tensor(out=y_sb, in0=y_sb, in1=mean_bc, op=ALU.subtract)
                y16 = yp.tile([128, NT, D], DT.float16, tag="y16")
                if apply_gamma_beta:
                    nc.vector.tensor_tensor(out=y_sb, in0=y_sb, in1=rstd_bc, op=ALU.mult)
                    for it in range(NT):
                        nc.gpsimd.tensor_mul(y_sb[:, it, :], y_sb[:, it, :], gam_sb)
                        nc.gpsimd.tensor_add(y_sb[:, it, :], y_sb[:, it, :], bet_sb)
                    nc.vector.tensor_copy(y16, y_sb)
                else:
                    nc.vector.tensor_tensor(out=y16, in0=y_sb, in1=rstd_bc, op=ALU.mult)
                nc.sync.dma_start(
                    out=out_t[b].rearrange("(t p) d -> p t d", p=128), in_=y16
                )

            TOTAL = BPC * NT
            for T in range(TOTAL + 2):
                if T < TOTAL:
                    b, jt = divmod(T, NT)
                    x_sb, xs_sb, wt, g_sb, v_sb = state[b]
                    asq = sqp.tile([128, N], DT.float16, tag="asq")
                    pr = ps_r.tile([128, N], DT.float32, tag="r")
                    for h in range(2):
                        nc.tensor.matmul(
                            pr[:, h * 512 : (h + 1) * 512],
                            lhsT=wt[h * D : (h + 1) * D,
                                    jt * 128 : (jt + 1) * 128],
                            rhs=g_sb[h * D : (h + 1) * D,
                                     h * 512 : (h + 1) * 512],
                            tile_position=(h * 64, 0),
                        )
                    if jt in DVE_SQ:
                        rf = sqp.tile([128, N], DT.float16, tag="rf")
                        nc.vector.tensor_copy(rf, pr)
                        nc.vector.tensor_mul(asq, rf, rf)
                    else:
                        nc.scalar.activation(asq, pr, AF.Square)
                    # Schraudolph: f16 bits = round(asq + 15360) -> 2^z
                    e_sb = ep.tile([128, N], DT.int16, tag="e")
                    nc.vector.tensor_scalar(
                        out=e_sb, in0=asq, scalar1=BITS_BIAS, scalar2=None,
                        op0=ALU.add,
                    )
                    e_tiles[T] = (e_sb, None)
                    if jt == 2 and b + 1 < BPC:
                        state[b + 1] = emit_loads(b + 1)
                Tb = T - 2
                if Tb >= 0:
                    b2, jt2 = divmod(Tb, NT)
                    if jt2 == 0:
                        pot_new = ps_ot.tile([DP, N], DT.float32, tag="ot", name=f"pot{b2}")
                        pots[b2] = pot_new
                    pot = pots[b2]
                    e_pr, _ = e_tiles.pop(Tb)
                    v_sb2 = state[b2][4]
                    for c in range(2):
                        nc.tensor.matmul(
                            pot[:, c * 512 : (c + 1) * 512],
                            lhsT=v_sb2[:, jt2, :],
                            rhs=e_pr[:, c * 512 : (c + 1) * 512].bitcast(
                                DT.float16
                            ),
                            start=(jt2 == 0),
                            stop=(jt2 == NT - 1),
                        )
                    if jt2 == NT - 1:
                        ep_ot(b2)
                    if b2 >= 1:
                        if jt2 == 1:
                            ep_transpose(b2 - 1)
                        elif jt2 == 3:
                            ep_stats(b2 - 1)
                        elif jt2 == 5:
                            ep_norm(b2 - 1)

            # last batch: per-group staged epilogue to shorten the tail chain
            bL = BPC - 1
            x_sb, xs_sb, wt, g_sb, v_sb = state[bL]
            st = epi[bL]
            ot_sb = st["ot"]
            y_sb = yp.tile([128, NT, D], DT.float32, tag="y")
            ros = statp.tile([128, NT, 2], DT.float32, tag="ros")
            mean = statp.tile([128, NT], DT.float32, tag="mean")
            ysq = statp.tile([128, NT, D], DT.float32, tag="ysq")
            var = statp.tile([128, NT], DT.float32, tag="var")
            msq = statp.tile([128, NT], DT.float32, tag="msq")
            rstd = statp.tile([128, NT], DT.float32, tag="rstd")
            iv = statp.tile([128, NT], DT.int32, tag="iv")
            t1 = statp.tile([128, NT], DT.float32, tag="t1")
            for grp in range(2):
                g_sl = slice(grp * 4, grp * 4 + 4)
                ptr = ps_r.tile([128, 4, DP], DT.float32, tag="r")
                for q in range(4):
                    it = grp * 4 + q
                    nc.tensor.transpose(
                        ptr[:, q, :],
                        ot_sb[:, it * 128 : (it + 1) * 128],
                        id_sb[0:DP, 0:DP],
                    )
                nc.vector.tensor_copy(ros[:, g_sl, :], ptr[:, :, D : D + 2])
                rs4 = ros[:, g_sl, 0]
                rs_bc = bass.AP(
                    tensor=ros.tensor, offset=rs4.offset,
                    ap=[ros.ap[0], [2, 4], [0, D]],
                )
                nc.vector.tensor_tensor(
                    out=y_sb[:, g_sl, :], in0=x_sb[:, g_sl, :], in1=rs_bc,
                    op=ALU.mult,
                )
                nc.vector.tensor_add(
                    y_sb[:, g_sl, :], y_sb[:, g_sl, :], ptr[:, :, 0:D]
                )
                nc.vector.tensor_mul(mean[:, g_sl], ros[:, g_sl, 0], xs_sb[:, g_sl])
                nc.vector.tensor_add(mean[:, g_sl], mean[:, g_sl], ros[:, g_sl, 1])
                nc.scalar.activation(ysq[:, g_sl, :], y_sb[:, g_sl, :], AF.Square)
                nc.vector.reduce_sum(
                    var[:, g_sl], ysq[:, g_sl, :], axis=mybir.AxisListType.X
                )
                nc.vector.tensor_scalar_mul(var[:, g_sl], var[:, g_sl], 1.0 / D)
                nc.vector.tensor_mul(msq[:, g_sl], mean[:, g_sl], mean[:, g_sl])
                nc.vector.tensor_sub(var[:, g_sl], var[:, g_sl], msq[:, g_sl])
                nc.vector.tensor_scalar(
                    iv[:, g_sl], var[:, g_sl].bitcast(DT.int32), scalar1=1,
                    scalar2=None, op0=ALU.arith_shift_right,
                )
                nc.vector.tensor_scalar(
                    iv[:, g_sl], iv[:, g_sl], scalar1=-1, scalar2=None,
                    op0=ALU.bitwise_xor,
                )
                nc.vector.tensor_scalar_add(iv[:, g_sl], iv[:, g_sl], 0x5F3759E0)
                yk = iv.bitcast(DT.float32)
                nc.vector.tensor_mul(t1[:, g_sl], yk[:, g_sl], yk[:, g_sl])
                nc.vector.tensor_mul(t1[:, g_sl], t1[:, g_sl], var[:, g_sl])
                nc.vector.tensor_scalar(
                    t1[:, g_sl], t1[:, g_sl], scalar1=-0.5, scalar2=1.5,
                    op0=ALU.mult, op1=ALU.add,
                )
                nc.vector.tensor_mul(rstd[:, g_sl], yk[:, g_sl], t1[:, g_sl])
                m4 = mean[:, g_sl]
                mean_bc = bass.AP(
                    tensor=mean.tensor, offset=m4.offset,
                    ap=[mean.ap[0], [1, 4], [0, D]],
                )
                r4 = rstd[:, g_sl]
                rstd_bc = bass.AP(
                    tensor=rstd.tensor, offset=r4.offset,
                    ap=[rstd.ap[0], [1, 4], [0, D]],
                )
                nc.vector.tensor_tensor(
                    out=y_sb[:, g_sl, :], in0=y_sb[:, g_sl, :], in1=mean_bc,
                    op=ALU.subtract,
                )
                if grp == 0:
                    y16t = yp.tile([128, NT, D], DT.float16, tag="y16")
                if apply_gamma_beta:
                    nc.vector.tensor_tensor(
                        out=y_sb[:, g_sl, :], in0=y_sb[:, g_sl, :], in1=rstd_bc,
                        op=ALU.mult,
                    )
                    for it in range(grp * 4, grp * 4 + 4):
                        nc.gpsimd.tensor_mul(y_sb[:, it, :], y_sb[:, it, :], gam_sb)
                        nc.gpsimd.tensor_add(y_sb[:, it, :], y_sb[:, it, :], bet_sb)
                    nc.vector.tensor_copy(y16t[:, g_sl, :], y_sb[:, g_sl, :])
                else:
                    nc.vector.tensor_tensor(
                        out=y16t[:, g_sl, :], in0=y_sb[:, g_sl, :], in1=rstd_bc,
                        op=ALU.mult,
                    )
                nc.sync.dma_start(
                    out=out_t[bL]
                    .rearrange("(t p) d -> p t d", p=128)[:, g_sl, :],
                    in_=y16t[:, g_sl, :],
                )


    nc.compile()
    return nc


_NC_CACHE: dict = {}


def kernel(input1, V_w, V_b, phi, ln_gamma, ln_beta, _trace=False):
    input1 = np.ascontiguousarray(np.asarray(input1, dtype=np.float32))
    V_w = np.asarray(V_w, dtype=np.float32)
    V_b = np.asarray(V_b, dtype=np.float32)
    phi = np.asarray(phi, dtype=np.float32)
    ln_gamma = np.asarray(ln_gamma, dtype=np.float32)
    ln_beta = np.asarray(ln_beta, dtype=np.float32)

    apply_gb = not (np.all(ln_gamma == 1.0) and np.all(ln_beta == 0.0))

    if apply_gb not in _NC_CACHE:
        _NC_CACHE[apply_gb] = _build_nc(apply_gb)
    nc = _NC_CACHE[apply_gb]

    O = _build_orthogonal(phi)
    ident = np.eye(128, dtype=np.float32)
    gb = np.ascontiguousarray(np.stack([ln_gamma, ln_beta]).astype(np.float32))

    # host prep: scaled rows W = x * ||x||^-1/2 * 8^-1/4 * C^1/4, transposed;
    # G = O^T W^T; Vt = [X Vw^T + Vb | 1 | rowmean]; per-row sums / D
    ss = (input1.astype(np.float64) ** 2).sum(-1)
    s = (ss ** -0.25 * 8.0 ** -0.25 * CSCALE ** 0.25).astype(np.float32)
    w = input1 * s[..., None]
    w16 = w.astype(np.float16)
    wt_full = w16.transpose(0, 2, 1)
    o16f = O.astype(np.float16).astype(np.float32)
    g_full = np.einsum(
        "bnd,de->ben", w16.astype(np.float32), o16f, optimize=True
    ).astype(np.float16)
    top = np.concatenate([g_full, wt_full], axis=2)          # [B, 64, 2N]
    gw_full = np.ascontiguousarray(np.concatenate([top, top], axis=1))
    v_full = np.empty((B, N, DP), np.float32)
    v_full[:, :, 0:D] = input1 @ V_w.T + V_b
    v_full[:, :, D] = 1.0
    v_full[:, :, D + 1] = v_full[:, :, 0:D].sum(-1) / D
    v_full = np.ascontiguousarray(v_full.astype(np.float16))
    xz_full = np.empty((B, N, D + 1), np.float32)
    xz_full[:, :, 0:D] = input1
    xz_full[:, :, D] = input1.sum(-1) / D
    xz_full = np.ascontiguousarray(xz_full)

    in_maps = []
    for c in range(NCORES):
        sl = slice(c * BPC, (c + 1) * BPC)
        in_maps.append(
            {
                "xz": xz_full[sl],
                "gw": gw_full[sl],
                "v": v_full[sl],
                "ident": ident,
                "gb": gb,
            }
        )

    res = bass_utils.run_bass_kernel_spmd(
        nc, in_maps, core_ids=list(range(NCORES)), trace=_trace
    )
    out = np.concatenate([res.results[c]["out"] for c in range(NCORES)], axis=0).astype(np.float32)
    if _trace:
        kernel._last_result = res
    return out
